# revision 30
# baseline (speedup 1.0000x reference)
"""GPT-2 attention (B=2, S=2048, D=1024, H=16) on 8 TRN2 NeuronCores.

Sharding: 2-way data parallel over batch x 4-way tensor parallel over heads.
Core c handles batch b = c // 4 and heads 4g..4g+3 where g = c % 4.

Per-core kernel (all matmul inputs bf16, fp32 PSUM accumulation):
  1. QKV^T projection: Q^T, K^T in [head_dim, seq] layout; the head pair
     (2j, 2j+1) of plane j lives at partitions 0-63 / 64-127. V in natural
     [seq, head_dim] layout with a ones column appended (softmax denom).
  2. Scores: per (chunk, plane, sk-tile): TWO K=64 matmuls (one per head of
     the pair) into adjacent PSUM banks. They occupy disjoint row groups of
     the PE array (partitions 0-63 vs 64-127) so the hardware runs them
     concurrently (~2x). Causal mask on diagonal tiles via an identity
     matmul accumulating a -1e9 upper-triangle tile. One exp per plane
     (a single exp across both PSUM banks is a fatal cross-bank ACT read).
  3. AV: O_aug^T[65, sq] = V_aug.T @ P^T accumulated over sk tiles; row 64
     is the softmax denominator.
  4. Normalization (baseline-proven path): the denominator row is bounced
     through DRAM to re-partition [1,512] -> [128,4] so the reciprocal runs
     on 128 lanes (DVE reciprocal costs ~8 cyc/elem/lane, so lane-spreading
     matters), broadcast across 64 partitions with a K=1 matmul, multiplied
     into O^T. Deferred several units so the chain latency stays hidden.
  5. Output projection: y_partial[sq, 1024] = O_scaled^T.T @ Wproj_shard,
     emitted one chunk behind attention.

The emission interleaves scores tiles with AV/production/projection work
through a FIFO "filler" so the PE queue never stalls on the scores->exp
(PSUM WAR) chain. Watermarks enforce emission-order prerequisites for
buffer reuse.

Host: x[b].T and weight shards pre-cast to bf16; the 4 per-batch partials
are summed on host (row-split matmul unshard) and bproj added.
"""

import numpy as np
import ml_dtypes

import concourse.bass as bass
import concourse.mybir as mybir
import concourse.tile as tile
from concourse import bacc
from concourse.bass_utils import run_bass_kernel_spmd
from concourse.masks import make_identity

BF16 = ml_dtypes.bfloat16

B, S, D, H = 2, 2048, 1024, 16
HD = D // H            # 64
NH = 4                 # heads per core
JQ = NH * HD           # 256 q (or k, or v) columns per core
P = 128
SC = 512               # seq chunk (matmul free dim / PSUM bank)
NSC = S // SC          # 4
NST = S // P           # 16 seq tiles
NDC = D // P           # 8 contraction chunks over model dim
SCALE = 1.0 / np.sqrt(np.float32(HD))  # 0.125
NEG = -1.0e9

_COMPILED = {}


def build(has_qkv_bias: bool):
    f32 = mybir.dt.float32
    bf16 = mybir.dt.bfloat16
    nc = bacc.Bacc()

    xT = nc.declare_dram_parameter("xT", [D, S], bf16, isOutput=False)
    w = nc.declare_dram_parameter("w", [D, 3 * JQ], bf16, isOutput=False)
    wp = nc.declare_dram_parameter("wp", [JQ, D], bf16, isOutput=False)
    if has_qkv_bias:
        bqkv = nc.declare_dram_parameter("bqkv", [1, 3 * JQ], bf16, isOutput=False)
    y = nc.declare_dram_parameter("y", [S, D], bf16, isOutput=True)

    with tile.TileContext(nc) as tc:
        with (
            tc.tile_pool(name="const", bufs=1) as const,
            tc.tile_pool(name="inp", bufs=1) as inp,
            tc.tile_pool(name="qkv", bufs=1) as qkv,
            tc.tile_pool(name="ptp", bufs=2) as ptp,
            tc.tile_pool(name="ps", bufs=2, space="PSUM") as psp,
            tc.tile_pool(name="drp", bufs=1, space="DRAM") as drp,
        ):
            # ---- constants ----
            if has_qkv_bias:
                ones = const.tile([P, SC], bf16)
                nc.gpsimd.memset(ones[:], 1.0)

            # ---- load inputs, split so early consumers start immediately ----
            # w tiles 0,1: Q columns; 2,3: K columns; V columns in one tile
            wt = [inp.tile([P, NDC, P], bf16, name=f"wt{j}") for j in range(4)]
            wv_sb = inp.tile([P, NDC, JQ], bf16)
            xc = [inp.tile([P, NDC, SC], bf16, name=f"xc{c}") for c in range(NSC)]

            def dma_w(j):
                nc.sync.dma_start(
                    out=wt[j][:],
                    in_=w[:, j * P:(j + 1) * P].rearrange("(a p) j2 -> p a j2", p=P),
                )

            def dma_x(c):
                nc.sync.dma_start(
                    out=xc[c][:],
                    in_=xT[:, c * SC:(c + 1) * SC].rearrange("(a p) s -> p a s", p=P),
                )

            dma_w(2)
            dma_w(0)
            nc.sync.dma_start(
                out=xc[0][:, 0:4, :],
                in_=xT[0:D // 2, 0:SC].rearrange("(a p) s -> p a s", p=P),
            )
            nc.sync.dma_start(
                out=xc[0][:, 4:NDC, :],
                in_=xT[D // 2:D, 0:SC].rearrange("(a p) s -> p a s", p=P),
            )
            nc.sync.dma_start(
                out=wv_sb[:],
                in_=w[:, 2 * JQ:3 * JQ].rearrange("(a p) j2 -> p a j2", p=P),
            )
            dma_w(3)
            dma_w(1)
            for c in range(1, NSC):
                dma_x(c)
            wp_sb = inp.tile([P, JQ // P, D], bf16)
            nc.sync.dma_start(out=wp_sb[:], in_=wp[:].rearrange("(a p) j -> p a j", p=P))
            if has_qkv_bias:
                b_sb = inp.tile([1, 3 * JQ], bf16)
                nc.sync.dma_start(out=b_sb[:], in_=bqkv[:])

            qT = qkv.tile([P, 2, S], bf16)  # partitions: head pair (h%2)*64 + hd
            kT = qkv.tile([P, 2, S], bf16)
            v_sb = qkv.tile([P, NST * NH, HD + 1], bf16)
            nc.vector.memset(v_sb[:, :, HD:HD + 1], 1.0)
            oT = qkv.tile([P, 2, S], bf16)
            dr_s = drp.tile([NH, NSC, 1, SC], f32)
            dr_r = drp.tile([NH, NSC, 1, SC], bf16)

            def emit_qk_chunk(jt, c):
                # one [128, SC] chunk of Q^T (jt 0,1) or K^T (jt 2,3)
                dest, jl = (qT, jt) if jt < 2 else (kT, jt - 2)
                ps_qkv = psp.tile([P, SC], f32, tag="ps", name="ps_qkv")
                for a in range(NDC):
                    nc.tensor.matmul(
                        ps_qkv[:],
                        lhsT=wt[jt][:, a, :],
                        rhs=xc[c][:, a, :],
                        start=(a == 0),
                        stop=(a == NDC - 1) and not has_qkv_bias,
                    )
                if has_qkv_bias:
                    nc.tensor.matmul(
                        ps_qkv[:],
                        lhsT=b_sb[0:1, jt * P:(jt + 1) * P],
                        rhs=ones[0:1, :SC],
                        start=False,
                        stop=True,
                    )
                nc.vector.tensor_copy(dest[:, jl, c * SC:(c + 1) * SC], ps_qkv[:])

            def emit_v_tile(t):
                # V rows for seq tile t, all 4 heads, with the ones column
                ps_v = psp.tile([P, SC], f32, tag="ps", name="ps_v")
                for a in range(NDC):
                    nc.tensor.matmul(
                        ps_v[:, 0:JQ],
                        lhsT=xc[t // 4][:, a, (t % 4) * P:(t % 4 + 1) * P],
                        rhs=wv_sb[:, a, :],
                        start=(a == 0),
                        stop=(a == NDC - 1) and not has_qkv_bias,
                    )
                if has_qkv_bias:
                    nc.tensor.matmul(
                        ps_v[:, 0:JQ],
                        lhsT=ones[0:1, 0:P],
                        rhs=b_sb[0:1, 2 * JQ:3 * JQ],
                        start=False,
                        stop=True,
                    )
                nc.vector.tensor_copy(
                    v_sb[:, t * NH:(t + 1) * NH, 0:HD],
                    ps_v[:, 0:JQ].rearrange("p (h d) -> p h d", d=HD),
                )

            # ---- deferred per-unit normalization, staged so the DVE queue
            # never head-of-line-blocks on an in-flight DMA: each stage is
            # emitted 1+ units after the DMAs it depends on were started.
            # Stage a (in epi): denom row -> DRAM -> [128,4] repack DMA.
            # Stage b (+1 unit): reciprocal on 128 lanes, DMA back to DRAM.
            # Stage c (+2): one DMA broadcasts the recip row across 64
            #   partitions (stride-0 view) -- no PE involvement at all.
            # Stage d (+3): DVE multiply into oT.
            rp_pend = []
            bc_pend = []
            mul_pend = []
            # chunk -> processing position, set once the schedule below
            # fixes the chunk order (used to flush "everything processed
            # up to and including chunk c")
            cpos = {}

            def emit_recip(e):
                h, c, rsc, rscb = e
                with nc.allow_low_precision(reason="bf16 softmax denom recip"):
                    nc.vector.reciprocal(rscb[:], rsc[:])
                nc.sync.dma_start(
                    out=dr_r[h, c].rearrange("x (p k) -> (x p) k", p=P),
                    in_=rscb[:],
                )
                bc_pend.append((h, c))

            def emit_bc(e):
                h, c = e
                po = (h % 2) * HD
                # full-height tile: the mul's SBUF inputs must share their
                # start partition (walrus checkSBSameStartPartition)
                rbc = ptp.tile([P, SC], bf16, tag="rbc", name="rbc", bufs=6)
                nc.sync.dma_start(
                    out=rbc[po:po + HD, :],
                    in_=dr_r[h, c].partition_broadcast(HD),
                )
                mul_pend.append((h, c, rbc))

            def emit_mul(e):
                h, c, rbc = e
                jl, po = h // 2, (h % 2) * HD
                nc.vector.tensor_mul(
                    oT[po:po + HD, jl, c * SC:(c + 1) * SC],
                    oT[po:po + HD, jl, c * SC:(c + 1) * SC],
                    rbc[po:po + HD, :],
                )

            def emit_mul_quarter(entry, q):
                # tail: normalize one sq sub-tile so its projection can
                # start before the rest of the chunk is normalized
                h, c, rbc = entry
                jl, po = h // 2, (h % 2) * HD
                lo = c * SC + q * P
                nc.vector.tensor_mul(
                    oT[po:po + HD, jl, lo:lo + P],
                    oT[po:po + HD, jl, lo:lo + P],
                    rbc[po:po + HD, q * P:(q + 1) * P],
                )

            _STAGES = ((rp_pend, emit_recip), (bc_pend, emit_bc),
                       (mul_pend, emit_mul))

            def flush_norm(keep_r, keep_b, keep_m):
                for (pend, emit), keep in zip(_STAGES,
                                              (keep_r, keep_b, keep_m)):
                    while len(pend) > keep:
                        emit(pend.pop(0))

            def flush_norm_through(cmax):
                pm = cpos[cmax]
                for pend, emit in _STAGES:
                    i = 0
                    while i < len(pend):
                        if cpos[pend[i][1]] <= pm:
                            emit(pend.pop(i))
                        else:
                            i += 1

            # ---- filler machinery ----
            filler_q = []
            n_queued = [0]
            n_emitted = [0]

            def qput(cost, th):
                filler_q.append((cost, th))
                n_queued[0] += 1
                return n_queued[0]

            def pump(budget):
                while budget > 0 and filler_q:
                    cost, th = filler_q.pop(0)
                    th()
                    n_emitted[0] += 1
                    budget -= cost

            def drain_to(wm):
                while n_emitted[0] < wm and filler_q:
                    _, th = filler_q.pop(0)
                    th()
                    n_emitted[0] += 1

            def drain():
                while filler_q:
                    filler_q.pop(0)[1]()
                    n_emitted[0] += 1

            def tile_order(c):
                # diagonal tiles first (descending), then off-diagonal:
                # their post-exp causal-mask selects run on GPSIMD with a
                # full pair of slack before the (also diag-first) AV pass
                # consumes them.
                return [4 * c + 3, 4 * c + 2, 4 * c + 1, 4 * c] + \
                    list(range(4 * c))

            def emit_scores_pair(c, jl, wm=0):
                # scores^T + exp for the head pair (2*jl, 2*jl+1)
                drain_to(wm)
                pt = ptp.tile([P, 2, NST, SC], bf16, tag="pt", name="pt")
                for t in tile_order(c):
                    coff = max(0, t * P - c * SC)
                    diag = t >= 4 * c
                    ps_sc = psp.tile([P, 2, SC], f32, tag="ps_sc",
                                     name="ps_sc", bufs=2)
                    for i in range(2):
                        nc.tensor.matmul(
                            ps_sc[:, i, coff:],
                            lhsT=kT[i * HD:(i + 1) * HD, jl, t * P:(t + 1) * P],
                            rhs=qT[i * HD:(i + 1) * HD, jl,
                                   c * SC + coff:(c + 1) * SC],
                            start=True,
                            stop=True,
                        )
                    for i in range(2):
                        nc.scalar.activation(
                            pt[:, i, t, coff:], ps_sc[:, i, coff:],
                            mybir.ActivationFunctionType.Exp, scale=float(SCALE),
                        )
                    if diag:
                        # zero exp of the masked (j < p) region, both planes
                        nc.gpsimd.affine_select(
                            out=pt[:, :, t, coff:coff + P],
                            in_=pt[:, :, t, coff:coff + P],
                            compare_op=mybir.AluOpType.is_ge,
                            fill=0.0,
                            base=0,
                            pattern=[[0, 2], [1, P]],
                            channel_multiplier=-1,
                        )
                    pump(1300)
                return (c, jl, pt)

            def queue_av_pair(state, direct_recip=False):
                # AV for both heads of the pair, as fine per-tile thunks.
                # Epilogue: evacuate O^T, extract the denominator row and
                # start its reciprocal chain; the normalization itself is
                # deferred via norm_pend.
                c, jl, pt = state
                ts = tile_order(c)
                wm = 0
                for i in range(2):
                    h = 2 * jl + i
                    box = {}

                    def mk_mm(k, i=i, h=h, box=box, c=c, ts=ts, pt=pt):
                        def th():
                            if "ps" not in box:
                                box["ps"] = psp.tile(
                                    [P, SC], f32, tag="ps_av",
                                    name="ps_av", bufs=2,
                                )
                            t = ts[k]
                            coff = max(0, t * P - c * SC)
                            nc.tensor.matmul(
                                box["ps"][0:HD + 1, coff:],
                                lhsT=v_sb[:, t * NH + h, :],
                                rhs=pt[:, i, t, coff:],
                                start=(k == 0),
                                stop=(k == len(ts) - 1),
                            )
                        return th

                    def mk_epi(i=i, h=h, box=box, c=c, jl=jl,
                               direct=direct_recip):
                        def th():
                            ps_av = box["ps"]
                            po = i * HD
                            if direct:
                                # tail units: single-lane reciprocal skips
                                # the [128,4] repack roundtrip; ScalarE
                                # (idle by now) evacuates O^T
                                rrow = ptp.tile([1, SC], bf16, tag="rrow",
                                                name="rrow", bufs=2)
                                with nc.allow_low_precision(
                                    reason="bf16 softmax denom recip"
                                ):
                                    nc.vector.reciprocal(
                                        rrow[:], ps_av[HD:HD + 1, :]
                                    )
                                nc.sync.dma_start(
                                    out=dr_r[h, c], in_=rrow[0:1, :]
                                )
                                nc.scalar.copy(
                                    oT[po:po + HD, jl, c * SC:(c + 1) * SC],
                                    ps_av[0:HD, :],
                                )
                                bc_pend.append((h, c))
                            else:
                                nc.vector.tensor_copy(
                                    oT[po:po + HD, jl, c * SC:(c + 1) * SC],
                                    ps_av[0:HD, :],
                                )
                                sumst = ptp.tile([P, SC], f32, tag="sumst",
                                                 name="sumst", bufs=3)
                                rsc = ptp.tile([P, SC // P], f32, tag="rsc",
                                               name="rsc", bufs=3)
                                rscb = ptp.tile([P, SC // P], bf16, tag="rscb",
                                                name="rscb", bufs=3)
                                nc.vector.tensor_copy(
                                    sumst[64:65, :], ps_av[HD:HD + 1, :]
                                )
                                nc.sync.dma_start(
                                    out=dr_s[h, c], in_=sumst[64:65, :]
                                )
                                nc.sync.dma_start(
                                    out=rsc[:],
                                    in_=dr_s[h, c].rearrange(
                                        "x (p k) -> (x p) k", p=P
                                    ),
                                )
                                rp_pend.append((h, c, rsc, rscb))
                            flush_norm(1, 1, 1)
                        return th

                    for k in range(len(ts)):
                        qput(260, mk_mm(k))
                    wm = qput(0, mk_epi())
                return wm

            def emit_proj(st, jc):
                # normalization of this chunk must be flushed first
                flush_norm_through(st // 4)
                ps_y = psp.tile([P, SC], f32, tag="ps", name="ps_y")
                for cc in range(2):
                    nc.tensor.matmul(
                        ps_y[:],
                        lhsT=oT[:, cc, st * P:(st + 1) * P],
                        rhs=wp_sb[:, cc, jc * SC:(jc + 1) * SC],
                        start=(cc == 0),
                        stop=(cc == 1),
                    )
                y_sb = ptp.tile([P, SC], bf16, tag="ysb", name="y_sb", bufs=4)
                if st >= 12:  # tail tiles: ScalarE is idle after the last exp
                    nc.scalar.copy(y_sb[:], ps_y[:])
                else:
                    nc.vector.tensor_copy(y_sb[:], ps_y[:])
                nc.sync.dma_start(
                    out=y[st * P:(st + 1) * P, jc * SC:(jc + 1) * SC], in_=y_sb[:]
                )

            def queue_prod(c):
                qput(1600, lambda: emit_qk_chunk(2, c))
                qput(1600, lambda: emit_qk_chunk(0, c))
                for t in range(4 * c, 4 * c + 4):
                    qput(1100, lambda t=t: emit_v_tile(t))
                qput(1600, lambda: emit_qk_chunk(3, c))
                return qput(1600, lambda: emit_qk_chunk(1, c))

            def queue_proj(c):
                for st in range(4 * c, 4 * c + 4):
                    for jc in range(2):
                        qput(500, lambda st=st, jc=jc: emit_proj(st, jc))

            # ---- main schedule ----
            # Chunk processing order [1, 2, 3, 0]: scores(c) needs qT(c)
            # and kT chunks <= c only, so the order is free up to
            # production. Starting with chunk 1 (needs just prod<=1) gets
            # the exp stream flowing early; production of later chunks
            # drains as filler during earlier chunks; the tiny chunk 0
            # runs last so the exposed end-of-kernel normalization chain
            # hangs off a 4-tile AV instead of a 16-tile one. qT(0) is
            # produced last - it isn't needed until the final pairs.
            emit_qk_chunk(2, 0)   # K plane 0, chunk 0
            emit_qk_chunk(2, 1)   # K plane 0, chunk 1
            emit_qk_chunk(0, 1)   # Q plane 0, chunk 1
            for t in range(8):
                qput(1100, lambda t=t: emit_v_tile(t))
            qput(1600, lambda: emit_qk_chunk(3, 0))
            qput(1600, lambda: emit_qk_chunk(3, 1))
            wm_p11 = qput(1600, lambda: emit_qk_chunk(1, 1))
            prod_wm = {1: wm_p11}
            for c in (2, 3):
                prod_wm[c] = queue_prod(c)
            qput(1600, lambda: emit_qk_chunk(0, 0))  # Q chunk 0, late
            prod_wm[0] = qput(1600, lambda: emit_qk_chunk(1, 0))

            corder = [1, 2, 3, 0]
            cpos.update({c: i for i, c in enumerate(corder)})
            pairs = [(c, jl) for c in corder for jl in range(2)]
            # proj(c) becomes available after pair (c,1)'s norms complete;
            # proj(0) runs in the tail.
            proj_slot = {2: 1, 4: 2, 6: 3}  # pair index m -> chunk to queue

            av_wm = {}
            for m, (c, jl) in enumerate(pairs):
                wm = 0 if (m, jl) == (0, 0) else prod_wm[c]
                if m >= 2:
                    wm = max(wm, av_wm[pairs[m - 2]])
                s = emit_scores_pair(c, jl, wm=wm)
                av_wm[(c, jl)] = queue_av_pair(s)
                if m in proj_slot:
                    queue_proj(proj_slot[m])
            drain()
            # tail: pop the last chunk's final pair for quartered muls,
            # flush the rest, then interleave per-tile muls with the last
            # chunk's projections
            clast = corder[-1]
            flush_norm(0, 0, 10 ** 9)
            tail = [e for e in mul_pend if e[1] == clast and e[0] >= 2]
            rest = [e for e in mul_pend if not (e[1] == clast and e[0] >= 2)]
            del mul_pend[:]
            mul_pend.extend(rest)
            flush_norm(0, 0, 0)
            for q in range(4):
                for e in tail:
                    emit_mul_quarter(e, q)
                for jc in range(2):
                    emit_proj(4 * clast + q, jc)

    nc.compile()
    return nc


def get_compiled(has_qkv_bias: bool):
    key = bool(has_qkv_bias)
    if key not in _COMPILED:
        _COMPILED[key] = build(key)
    return _COMPILED[key]


def make_in_maps(x, Wqkv, bqkv, Wproj):
    has_bias = bool(np.any(bqkv))
    xTs = [np.ascontiguousarray(x[b].T).astype(BF16) for b in range(B)]
    in_maps = []
    for c in range(8):
        b, g = c // 4, c % 4
        sl = slice(g * JQ, (g + 1) * JQ)
        wshard = np.concatenate(
            [Wqkv[:, sl], Wqkv[:, D + g * JQ:D + (g + 1) * JQ],
             Wqkv[:, 2 * D + g * JQ:2 * D + (g + 1) * JQ]], axis=1
        ).astype(BF16)
        m = {
            "xT": xTs[b],
            "w": np.ascontiguousarray(wshard),
            "wp": np.ascontiguousarray(Wproj[sl]).astype(BF16),
        }
        if has_bias:
            bshard = np.concatenate(
                [bqkv[sl], bqkv[D + g * JQ:D + (g + 1) * JQ],
                 bqkv[2 * D + g * JQ:2 * D + (g + 1) * JQ]]
            ).astype(BF16)
            m["bqkv"] = np.ascontiguousarray(bshard[None, :])
        in_maps.append(m)
    return has_bias, in_maps


def run(x, Wqkv, bqkv, Wproj, bproj, trace=False):
    has_bias, in_maps = make_in_maps(x, Wqkv, bqkv, Wproj)
    nc = get_compiled(has_bias)
    res = run_bass_kernel_spmd(nc, in_maps, core_ids=list(range(8)), trace=trace)
    out = np.zeros((B, S, D), np.float32)
    for c in range(8):
        out[c // 4] += res.results[c]["y"].astype(np.float32)
    out += bproj.astype(np.float32)
    return out, res


def kernel(x, Wqkv, bqkv, Wproj, bproj):
    x = np.asarray(x, np.float32)
    Wqkv = np.asarray(Wqkv, np.float32)
    bqkv = np.asarray(bqkv, np.float32)
    Wproj = np.asarray(Wproj, np.float32)
    out, _ = run(x, Wqkv, bqkv, Wproj, bproj, trace=False)
    return out


# revision 38
# speedup vs baseline: 1.0398x; 1.0398x over previous
"""GPT-2 attention (B=2, S=2048, D=1024, H=16) on 8 TRN2 NeuronCores.

Sharding: 2-way data parallel over batch x 4-way tensor parallel over heads.
Core c handles batch b = c // 4 and heads 4g..4g+3 where g = c % 4.

Per-core kernel (all matmul inputs bf16, fp32 PSUM accumulation):
  1. QKV^T projection: Q^T, K^T in [head_dim, seq] layout; the head pair
     (2j, 2j+1) of plane j lives at partitions 0-63 / 64-127. V in natural
     [seq, head_dim] layout with a ones column appended (softmax denom).
  2. Scores: per (chunk, plane, sk-tile): TWO K=64 matmuls (one per head of
     the pair) into adjacent PSUM banks. They occupy disjoint row groups of
     the PE array (partitions 0-63 vs 64-127) so the hardware runs them
     concurrently (~2x). Causal mask on diagonal tiles via an identity
     matmul accumulating a -1e9 upper-triangle tile. One exp per plane
     (a single exp across both PSUM banks is a fatal cross-bank ACT read).
  3. AV: O_aug^T[65, sq] = V_aug.T @ P^T accumulated over sk tiles; row 64
     is the softmax denominator.
  4. Normalization (baseline-proven path): the denominator row is bounced
     through DRAM to re-partition [1,512] -> [128,4] so the reciprocal runs
     on 128 lanes (DVE reciprocal costs ~8 cyc/elem/lane, so lane-spreading
     matters), broadcast across 64 partitions with a K=1 matmul, multiplied
     into O^T. Deferred several units so the chain latency stays hidden.
  5. Output projection: y_partial[sq, 1024] = O_scaled^T.T @ Wproj_shard,
     emitted one chunk behind attention.

The emission interleaves scores tiles with AV/production/projection work
through a FIFO "filler" so the PE queue never stalls on the scores->exp
(PSUM WAR) chain. Watermarks enforce emission-order prerequisites for
buffer reuse.

Host: x[b].T and weight shards pre-cast to bf16; the 4 per-batch partials
are summed on host (row-split matmul unshard) and bproj added.
"""

import numpy as np
import ml_dtypes

import concourse.bass as bass
import concourse.mybir as mybir
import concourse.tile as tile
from concourse import bacc
from concourse.bass_utils import run_bass_kernel_spmd
from concourse.masks import make_identity

BF16 = ml_dtypes.bfloat16

B, S, D, H = 2, 2048, 1024, 16
HD = D // H            # 64
NH = 4                 # heads per core
JQ = NH * HD           # 256 q (or k, or v) columns per core
P = 128
SC = 512               # seq chunk (matmul free dim / PSUM bank)
NSC = S // SC          # 4
NST = S // P           # 16 seq tiles
NDC = D // P           # 8 contraction chunks over model dim
SCALE = 1.0 / np.sqrt(np.float32(HD))  # 0.125
NEG = -1.0e9

_COMPILED = {}


def build(has_qkv_bias: bool):
    f32 = mybir.dt.float32
    bf16 = mybir.dt.bfloat16
    nc = bacc.Bacc()

    xT = nc.declare_dram_parameter("xT", [D, S], bf16, isOutput=False)
    w = nc.declare_dram_parameter("w", [D, 3 * JQ], bf16, isOutput=False)
    wp = nc.declare_dram_parameter("wp", [JQ, D], bf16, isOutput=False)
    if has_qkv_bias:
        bqkv = nc.declare_dram_parameter("bqkv", [1, 3 * JQ], bf16, isOutput=False)
    y = nc.declare_dram_parameter("y", [S, D], bf16, isOutput=True)

    with tile.TileContext(nc) as tc:
        with (
            tc.tile_pool(name="const", bufs=1) as const,
            tc.tile_pool(name="inp", bufs=1) as inp,
            tc.tile_pool(name="qkv", bufs=1) as qkv,
            tc.tile_pool(name="ptp", bufs=2) as ptp,
            tc.tile_pool(name="ps", bufs=2, space="PSUM") as psp,
            tc.tile_pool(name="drp", bufs=1, space="DRAM") as drp,
        ):
            # ---- constants ----
            if has_qkv_bias:
                ones = const.tile([P, SC], bf16)
                nc.gpsimd.memset(ones[:], 1.0)

            # ---- load inputs, split so early consumers start immediately ----
            # w tiles 0,1: Q columns; 2,3: K columns; V columns in one tile
            wt = [inp.tile([P, NDC, P], bf16, name=f"wt{j}") for j in range(4)]
            wv_sb = inp.tile([P, NDC, JQ], bf16)
            xc = [inp.tile([P, NDC, SC], bf16, name=f"xc{c}") for c in range(NSC)]

            def dma_w(j):
                nc.sync.dma_start(
                    out=wt[j][:],
                    in_=w[:, j * P:(j + 1) * P].rearrange("(a p) j2 -> p a j2", p=P),
                )

            def dma_x(c):
                nc.sync.dma_start(
                    out=xc[c][:],
                    in_=xT[:, c * SC:(c + 1) * SC].rearrange("(a p) s -> p a s", p=P),
                )

            # order matches the chunk-[1,2,3,0] prologue: the first three
            # qk chunks need wt2 + xc0 + xc1 + wt0
            dma_w(2)
            nc.sync.dma_start(
                out=xc[0][:, 0:4, :],
                in_=xT[0:D // 2, 0:SC].rearrange("(a p) s -> p a s", p=P),
            )
            nc.sync.dma_start(
                out=xc[0][:, 4:NDC, :],
                in_=xT[D // 2:D, 0:SC].rearrange("(a p) s -> p a s", p=P),
            )
            dma_x(1)
            dma_w(0)
            nc.sync.dma_start(
                out=wv_sb[:],
                in_=w[:, 2 * JQ:3 * JQ].rearrange("(a p) j2 -> p a j2", p=P),
            )
            dma_w(3)
            dma_w(1)
            for c in range(2, NSC):
                dma_x(c)
            wp_sb = inp.tile([P, JQ // P, D], bf16)
            nc.sync.dma_start(out=wp_sb[:], in_=wp[:].rearrange("(a p) j -> p a j", p=P))
            if has_qkv_bias:
                b_sb = inp.tile([1, 3 * JQ], bf16)
                nc.sync.dma_start(out=b_sb[:], in_=bqkv[:])

            qT = qkv.tile([P, 2, S], bf16)  # partitions: head pair (h%2)*64 + hd
            kT = qkv.tile([P, 2, S], bf16)
            v_sb = qkv.tile([P, NST * NH, HD + 1], bf16)
            nc.vector.memset(v_sb[:, :, HD:HD + 1], 1.0)
            oT = qkv.tile([P, 2, S], bf16)
            dr_s = drp.tile([NH, NSC, 1, SC], f32)
            dr_r = drp.tile([NH, NSC, 1, SC], bf16)

            def emit_qk_chunk(jt, c):
                # one [128, SC] chunk of Q^T (jt 0,1) or K^T (jt 2,3)
                dest, jl = (qT, jt) if jt < 2 else (kT, jt - 2)
                ps_qkv = psp.tile([P, SC], f32, tag="ps", name="ps_qkv")
                for a in range(NDC):
                    nc.tensor.matmul(
                        ps_qkv[:],
                        lhsT=wt[jt][:, a, :],
                        rhs=xc[c][:, a, :],
                        start=(a == 0),
                        stop=(a == NDC - 1) and not has_qkv_bias,
                    )
                if has_qkv_bias:
                    nc.tensor.matmul(
                        ps_qkv[:],
                        lhsT=b_sb[0:1, jt * P:(jt + 1) * P],
                        rhs=ones[0:1, :SC],
                        start=False,
                        stop=True,
                    )
                nc.vector.tensor_copy(dest[:, jl, c * SC:(c + 1) * SC], ps_qkv[:])

            def emit_v_tile(t):
                # V rows for seq tile t, all 4 heads, with the ones column
                ps_v = psp.tile([P, SC], f32, tag="ps", name="ps_v")
                for a in range(NDC):
                    nc.tensor.matmul(
                        ps_v[:, 0:JQ],
                        lhsT=xc[t // 4][:, a, (t % 4) * P:(t % 4 + 1) * P],
                        rhs=wv_sb[:, a, :],
                        start=(a == 0),
                        stop=(a == NDC - 1) and not has_qkv_bias,
                    )
                if has_qkv_bias:
                    nc.tensor.matmul(
                        ps_v[:, 0:JQ],
                        lhsT=ones[0:1, 0:P],
                        rhs=b_sb[0:1, 2 * JQ:3 * JQ],
                        start=False,
                        stop=True,
                    )
                nc.vector.tensor_copy(
                    v_sb[:, t * NH:(t + 1) * NH, 0:HD],
                    ps_v[:, 0:JQ].rearrange("p (h d) -> p h d", d=HD),
                )

            # ---- deferred per-unit normalization, staged so the DVE queue
            # never head-of-line-blocks on an in-flight DMA: each stage is
            # emitted 1+ units after the DMAs it depends on were started.
            # Stage a (in epi): denom row -> DRAM -> [128,4] repack DMA.
            # Stage b (+1 unit): reciprocal on 128 lanes, DMA back to DRAM.
            # Stage c (+2): one DMA broadcasts the recip row across 64
            #   partitions (stride-0 view) -- no PE involvement at all.
            # Stage d (+3): DVE multiply into oT.
            rp_pend = []
            bc_pend = []
            mul_pend = []
            # chunk -> processing position, set once the schedule below
            # fixes the chunk order (used to flush "everything processed
            # up to and including chunk c")
            cpos = {}

            def emit_recip(e):
                h, c, rsc, rscb = e
                with nc.allow_low_precision(reason="bf16 softmax denom recip"):
                    nc.vector.reciprocal(rscb[:], rsc[:])
                nc.sync.dma_start(
                    out=dr_r[h, c].rearrange("x (p k) -> (x p) k", p=P),
                    in_=rscb[:],
                )
                bc_pend.append((h, c))

            def emit_bc(e):
                h, c = e
                po = (h % 2) * HD
                # full-height tile: the mul's SBUF inputs must share their
                # start partition (walrus checkSBSameStartPartition)
                rbc = ptp.tile([P, SC], bf16, tag="rbc", name="rbc", bufs=6)
                nc.sync.dma_start(
                    out=rbc[po:po + HD, :],
                    in_=dr_r[h, c].partition_broadcast(HD),
                )
                mul_pend.append((h, c, rbc))

            def emit_mul(e):
                h, c, rbc = e
                jl, po = h // 2, (h % 2) * HD
                nc.vector.tensor_mul(
                    oT[po:po + HD, jl, c * SC:(c + 1) * SC],
                    oT[po:po + HD, jl, c * SC:(c + 1) * SC],
                    rbc[po:po + HD, :],
                )

            def emit_mul_quarter(entry, q):
                # tail: normalize one sq sub-tile so its projection can
                # start before the rest of the chunk is normalized
                h, c, rbc = entry
                jl, po = h // 2, (h % 2) * HD
                lo = c * SC + q * P
                nc.vector.tensor_mul(
                    oT[po:po + HD, jl, lo:lo + P],
                    oT[po:po + HD, jl, lo:lo + P],
                    rbc[po:po + HD, q * P:(q + 1) * P],
                )

            _STAGES = ((rp_pend, emit_recip), (bc_pend, emit_bc),
                       (mul_pend, emit_mul))

            def flush_norm(keep_r, keep_b, keep_m):
                for (pend, emit), keep in zip(_STAGES,
                                              (keep_r, keep_b, keep_m)):
                    while len(pend) > keep:
                        emit(pend.pop(0))

            def flush_norm_through(cmax):
                pm = cpos[cmax]
                for pend, emit in _STAGES:
                    i = 0
                    while i < len(pend):
                        if cpos[pend[i][1]] <= pm:
                            emit(pend.pop(i))
                        else:
                            i += 1

            # ---- filler machinery: two priority tiers ----
            # crit: AV matmuls + epilogues (the attention pipeline itself,
            # must track the scores stream with ~1-pair lag). bulk: QKV
            # production and projections (deadline is only the watermark).
            # Entries carry global sequence ids so watermarks ("everything
            # queued before X must be emitted") work across both tiers.
            critq = []
            bulkq = []
            n_seq = [0]

            def qput(cost, th, crit=False):
                n_seq[0] += 1
                (critq if crit else bulkq).append((n_seq[0], cost, th))
                return n_seq[0]

            def pump(budget):
                # crit first (up to ~60% of the budget), then bulk
                cb = budget * 6 // 10
                while cb > 0 and critq:
                    _, cost, th = critq.pop(0)
                    th()
                    cb -= cost
                    budget -= cost
                while budget > 0 and bulkq:
                    _, cost, th = bulkq.pop(0)
                    th()
                    budget -= cost
                while budget > 0 and critq:
                    _, cost, th = critq.pop(0)
                    th()
                    budget -= cost

            def _pop_next(wm):
                # pop the globally-oldest entry (seq order across tiers)
                cs = critq[0][0] if critq else None
                bs = bulkq[0][0] if bulkq else None
                if cs is None and bs is None:
                    return False
                if bs is None or (cs is not None and cs < bs):
                    if wm is not None and cs > wm:
                        return False
                    critq.pop(0)[2]()
                else:
                    if wm is not None and bs > wm:
                        return False
                    bulkq.pop(0)[2]()
                return True

            def drain_to(wm):
                while _pop_next(wm):
                    pass

            def drain():
                while _pop_next(None):
                    pass

            def tile_order(c):
                # diagonal tiles first (descending), then off-diagonal:
                # their post-exp causal-mask selects run on GPSIMD with a
                # full pair of slack before the (also diag-first) AV pass
                # consumes them.
                return [4 * c + 3, 4 * c + 2, 4 * c + 1, 4 * c] + \
                    list(range(4 * c))

            def emit_scores_pair(c, jl, wm=0):
                # scores^T + exp for the head pair (2*jl, 2*jl+1)
                drain_to(wm)
                pt = ptp.tile([P, 2, NST, SC], bf16, tag="pt", name="pt")
                for t in tile_order(c):
                    coff = max(0, t * P - c * SC)
                    diag = t >= 4 * c
                    ps_sc = psp.tile([P, 2, SC], f32, tag="ps_sc",
                                     name="ps_sc", bufs=2)
                    for i in range(2):
                        nc.tensor.matmul(
                            ps_sc[:, i, coff:],
                            lhsT=kT[i * HD:(i + 1) * HD, jl, t * P:(t + 1) * P],
                            rhs=qT[i * HD:(i + 1) * HD, jl,
                                   c * SC + coff:(c + 1) * SC],
                            start=True,
                            stop=True,
                        )
                    for i in range(2):
                        nc.scalar.activation(
                            pt[:, i, t, coff:], ps_sc[:, i, coff:],
                            mybir.ActivationFunctionType.Exp, scale=float(SCALE),
                        )
                    if diag:
                        # zero exp of the masked (j < p) region, both planes
                        nc.gpsimd.affine_select(
                            out=pt[:, :, t, coff:coff + P],
                            in_=pt[:, :, t, coff:coff + P],
                            compare_op=mybir.AluOpType.is_ge,
                            fill=0.0,
                            base=0,
                            pattern=[[0, 2], [1, P]],
                            channel_multiplier=-1,
                        )
                    pump(1300)
                return (c, jl, pt)

            def queue_av_pair(state, direct_recip=False):
                # AV for both heads of the pair, as fine per-tile thunks
                # in the crit tier. The V tiles it reads (bulk tier) must
                # be emitted first: drain to their watermark.
                c, jl, pt = state
                drain_to(v_wm[c])
                nv = min(4 * (c + 1), NST)
                wm = 0
                for i in range(2):
                    h = 2 * jl + i
                    box = {}

                    def mk_mm(t, i=i, h=h, box=box, c=c, nv=nv, pt=pt):
                        def th():
                            if "ps" not in box:
                                box["ps"] = psp.tile(
                                    [P, SC], f32, tag="ps_av",
                                    name="ps_av", bufs=2,
                                )
                            coff = max(0, t * P - c * SC)
                            nc.tensor.matmul(
                                box["ps"][0:HD + 1, coff:],
                                lhsT=v_sb[:, t * NH + h, :],
                                rhs=pt[:, i, t, coff:],
                                start=(t == 0),
                                stop=(t == nv - 1),
                            )
                        return th

                    def mk_epi(i=i, h=h, box=box, c=c, jl=jl,
                               direct=direct_recip):
                        def th():
                            ps_av = box["ps"]
                            po = i * HD
                            if direct:
                                # tail units: single-lane reciprocal skips
                                # the [128,4] repack roundtrip; ScalarE
                                # (idle by now) evacuates O^T
                                rrow = ptp.tile([1, SC], bf16, tag="rrow",
                                                name="rrow", bufs=2)
                                with nc.allow_low_precision(
                                    reason="bf16 softmax denom recip"
                                ):
                                    nc.vector.reciprocal(
                                        rrow[:], ps_av[HD:HD + 1, :]
                                    )
                                nc.sync.dma_start(
                                    out=dr_r[h, c], in_=rrow[0:1, :]
                                )
                                nc.scalar.copy(
                                    oT[po:po + HD, jl, c * SC:(c + 1) * SC],
                                    ps_av[0:HD, :],
                                )
                                bc_pend.append((h, c))
                            else:
                                nc.vector.tensor_copy(
                                    oT[po:po + HD, jl, c * SC:(c + 1) * SC],
                                    ps_av[0:HD, :],
                                )
                                sumst = ptp.tile([P, SC], f32, tag="sumst",
                                                 name="sumst", bufs=3)
                                rsc = ptp.tile([P, SC // P], f32, tag="rsc",
                                               name="rsc", bufs=3)
                                rscb = ptp.tile([P, SC // P], bf16, tag="rscb",
                                                name="rscb", bufs=3)
                                nc.vector.tensor_copy(
                                    sumst[64:65, :], ps_av[HD:HD + 1, :]
                                )
                                nc.sync.dma_start(
                                    out=dr_s[h, c], in_=sumst[64:65, :]
                                )
                                nc.sync.dma_start(
                                    out=rsc[:],
                                    in_=dr_s[h, c].rearrange(
                                        "x (p k) -> (x p) k", p=P
                                    ),
                                )
                                rp_pend.append((h, c, rsc, rscb))
                            flush_norm(1, 1, 1)
                        return th

                    for t in range(nv):
                        qput(260, mk_mm(t), crit=True)
                    wm = qput(0, mk_epi(), crit=True)
                return wm

            def emit_proj(st, jc):
                # normalization of this chunk must be flushed first
                flush_norm_through(st // 4)
                ps_y = psp.tile([P, SC], f32, tag="ps", name="ps_y")
                for cc in range(2):
                    nc.tensor.matmul(
                        ps_y[:],
                        lhsT=oT[:, cc, st * P:(st + 1) * P],
                        rhs=wp_sb[:, cc, jc * SC:(jc + 1) * SC],
                        start=(cc == 0),
                        stop=(cc == 1),
                    )
                y_sb = ptp.tile([P, SC], bf16, tag="ysb", name="y_sb", bufs=4)
                if st >= 12:  # tail tiles: ScalarE is idle after the last exp
                    nc.scalar.copy(y_sb[:], ps_y[:])
                else:
                    nc.vector.tensor_copy(y_sb[:], ps_y[:])
                nc.sync.dma_start(
                    out=y[st * P:(st + 1) * P, jc * SC:(jc + 1) * SC], in_=y_sb[:]
                )

            v_wm = {}

            def queue_prod(c):
                qput(1600, lambda: emit_qk_chunk(2, c))
                qput(1600, lambda: emit_qk_chunk(0, c))
                for t in range(4 * c, 4 * c + 4):
                    v_wm[c] = qput(1100, lambda t=t: emit_v_tile(t))
                qput(1600, lambda: emit_qk_chunk(3, c))
                return qput(1600, lambda: emit_qk_chunk(1, c))

            def queue_proj(c):
                for st in range(4 * c, 4 * c + 4):
                    for jc in range(2):
                        qput(500, lambda st=st, jc=jc: emit_proj(st, jc))

            # ---- main schedule ----
            # Chunk processing order [1, 2, 3, 0]: scores(c) needs qT(c)
            # and kT chunks <= c only, so the order is free up to
            # production. Starting with chunk 1 (needs just prod<=1) gets
            # the exp stream flowing early; production of later chunks
            # drains as filler during earlier chunks; the tiny chunk 0
            # runs last so the exposed end-of-kernel normalization chain
            # hangs off a 4-tile AV instead of a 16-tile one. qT(0) is
            # produced last - it isn't needed until the final pairs.
            emit_qk_chunk(2, 0)   # K plane 0, chunk 0
            emit_qk_chunk(2, 1)   # K plane 0, chunk 1
            emit_qk_chunk(0, 1)   # Q plane 0, chunk 1
            for t in range(8):
                v_wm[t // 4] = qput(1100, lambda t=t: emit_v_tile(t))
            qput(1600, lambda: emit_qk_chunk(3, 0))
            qput(1600, lambda: emit_qk_chunk(3, 1))
            wm_p11 = qput(1600, lambda: emit_qk_chunk(1, 1))
            prod_wm = {1: wm_p11}
            for c in (2, 3):
                prod_wm[c] = queue_prod(c)
            qput(1600, lambda: emit_qk_chunk(0, 0))  # Q chunk 0, late
            prod_wm[0] = qput(1600, lambda: emit_qk_chunk(1, 0))

            corder = [1, 2, 3, 0]
            cpos.update({c: i for i, c in enumerate(corder)})
            pairs = [(c, jl) for c in corder for jl in range(2)]
            # proj(c) becomes available after pair (c,1)'s norms complete;
            # proj(0) runs in the tail.
            proj_slot = {2: 1, 4: 2, 6: 3}  # pair index m -> chunk to queue

            av_wm = {}
            for m, (c, jl) in enumerate(pairs):
                wm = 0 if (m, jl) == (0, 0) else prod_wm[c]
                if m >= 2:
                    wm = max(wm, av_wm[pairs[m - 2]])
                s = emit_scores_pair(c, jl, wm=wm)
                av_wm[(c, jl)] = queue_av_pair(s)
                if m in proj_slot:
                    queue_proj(proj_slot[m])
            drain()
            # tail: pop the last chunk's final pair for quartered muls,
            # flush the rest, then interleave per-tile muls with the last
            # chunk's projections
            clast = corder[-1]
            flush_norm(0, 0, 10 ** 9)
            tail = [e for e in mul_pend if e[1] == clast and e[0] >= 2]
            rest = [e for e in mul_pend if not (e[1] == clast and e[0] >= 2)]
            del mul_pend[:]
            mul_pend.extend(rest)
            flush_norm(0, 0, 0)
            for q in range(4):
                for e in tail:
                    emit_mul_quarter(e, q)
                for jc in range(2):
                    emit_proj(4 * clast + q, jc)

    nc.compile()
    return nc


def get_compiled(has_qkv_bias: bool):
    key = bool(has_qkv_bias)
    if key not in _COMPILED:
        _COMPILED[key] = build(key)
    return _COMPILED[key]


def make_in_maps(x, Wqkv, bqkv, Wproj):
    has_bias = bool(np.any(bqkv))
    xTs = [np.ascontiguousarray(x[b].T).astype(BF16) for b in range(B)]
    in_maps = []
    for c in range(8):
        b, g = c // 4, c % 4
        sl = slice(g * JQ, (g + 1) * JQ)
        wshard = np.concatenate(
            [Wqkv[:, sl], Wqkv[:, D + g * JQ:D + (g + 1) * JQ],
             Wqkv[:, 2 * D + g * JQ:2 * D + (g + 1) * JQ]], axis=1
        ).astype(BF16)
        m = {
            "xT": xTs[b],
            "w": np.ascontiguousarray(wshard),
            "wp": np.ascontiguousarray(Wproj[sl]).astype(BF16),
        }
        if has_bias:
            bshard = np.concatenate(
                [bqkv[sl], bqkv[D + g * JQ:D + (g + 1) * JQ],
                 bqkv[2 * D + g * JQ:2 * D + (g + 1) * JQ]]
            ).astype(BF16)
            m["bqkv"] = np.ascontiguousarray(bshard[None, :])
        in_maps.append(m)
    return has_bias, in_maps


def run(x, Wqkv, bqkv, Wproj, bproj, trace=False):
    has_bias, in_maps = make_in_maps(x, Wqkv, bqkv, Wproj)
    nc = get_compiled(has_bias)
    res = run_bass_kernel_spmd(nc, in_maps, core_ids=list(range(8)), trace=trace)
    out = np.zeros((B, S, D), np.float32)
    for c in range(8):
        out[c // 4] += res.results[c]["y"].astype(np.float32)
    out += bproj.astype(np.float32)
    return out, res


def kernel(x, Wqkv, bqkv, Wproj, bproj):
    x = np.asarray(x, np.float32)
    Wqkv = np.asarray(Wqkv, np.float32)
    bqkv = np.asarray(bqkv, np.float32)
    Wproj = np.asarray(Wproj, np.float32)
    out, _ = run(x, Wqkv, bqkv, Wproj, bproj, trace=False)
    return out
